# revision 39
# baseline (speedup 1.0000x reference)
"""LongcatMoe Trainium2 kernel — expert-parallel sparse MoE across 8 NeuronCores.

Strategy (expert-parallel, fp8 DoubleRow):
  - Host computes the tiny router (fp64 softmax/top-k) and dispatches tokens
    by top-k expert id: core e receives the tokens routed to expert e
    (capacity C=256; the few overflow tokens fall back to an exact host
    computation), plus expert e's weights quantized to fp8 e4m3.
  - Each core runs the silu-gated MLP for its expert with fp8 DoubleRow
    matmuls (2x PE throughput, contraction 256 per instruction):
      pg = sum_m (64 Wg)^T x        [I-tile, C] psum, = 64 g
      sg = silu(pg / 64)            ACT engine
      mid = (pu/16) * sg  -> fp8    DVE scalar_tensor_tensor, = 4 mid
      pd = sum_j (64 Wd)^T mid      = 256 d
      y  = bf16(pd)                 DMA out, host divides by 256
  - Host combines: out[tok] += (gate_w/256) * y, plus the zero-expert
    (identity) term zero_w[t] * x[t], both in fp64.

Scales: weights x64 (std 0.02 -> 1.28 keeps e4m3 normals), x unscaled,
mid x4 (max |4 mid| ~ 47 << 240 = e4m3 max). All scale factors are exact
powers of two and are undone in the host combine.

SBUF layouts (per-partition contiguous DMAs, big rows to keep the DMA
engines descriptor-efficient; every trigger costs ~730ns of sequencer time;
all input triggers ride ONE ring in strict consumption order so the early
critical tensors get the full HBM bandwidth, ~342 GB/s sustained):
  xT  [128, HO, C]         xT[p, ho, t] = q(x[idx[t], ho*128+p]) (4 DMAs)
  wgc [8, 128, 2, IO, 128] gate weights per h-tile-pair, 2KB rows (8 DMAs)
  wuc same for up (8 DMAs)
  wd4 [4, 128, 4, IO, 128] down tiles in quarters, 4KB rows (4 DMAs)
  y2  [8, 128, 2, C] bf16 output pairs (= 256 * down), 1KB rows (8 DMAs)

Phase 1 runs m-major over gate (1A) then up (1B) with 8 PSUM accumulators,
one full bank each, under proper start/stop accumulation groups (HW PSUM
zeroing is bank-granular — one live group per bank). The PE starts as soon
as the first gate chunk + x chunk land (~0.4 MB), and phase 1B reuses each
bank right after its silu read. Phase 2's pd banks rotate through the same
pool, PSUM->SBUF casts alternate ACT/DVE, and output DMA triggers alternate
between the two otherwise-idle rings (GpSimd, SP).
"""

import os

import numpy as np
import ml_dtypes

T, H, I, E, Z, TOPK = 1024, 2048, 1024, 8, 8, 4
ROUTED_SCALING = 1.0
N_CORES = 8
P = 128
HO = H // P  # 16
IO = I // P  # 8
C = 256      # per-expert device capacity; overflow handled on host
SW = 64.0    # weight quantization scale
SM = 4.0     # mid quantization scale

_PROGRAM = None
LAST_RESULTS = None  # BassKernelResults of the most recent run (for test harness)
ACT_FUNC = "Silu"   # overridden to "Sigmoid" by the CoreSim test (no Silu there)


def _build_program():
    import concourse.mybir as mybir
    import concourse.tile as tile
    from concourse import bacc

    f32 = mybir.dt.float32
    bf16 = mybir.dt.bfloat16
    fp8 = mybir.dt.float8e4
    SILU = getattr(mybir.ActivationFunctionType, ACT_FUNC)
    DR = mybir.MatmulPerfMode.DoubleRow
    MUL = mybir.AluOpType.mult

    nc = bacc.Bacc(
        "TRN2",
        target_bir_lowering=False,
        debug=False,
        enable_asserts=False,
        num_devices=N_CORES,
    )
    COPY = mybir.ActivationFunctionType.Copy
    MP = HO // 2  # 8 h-tile pairs
    xT = nc.dram_tensor("xT", [P, HO, C], fp8, kind="ExternalInput").ap()
    wgc = nc.dram_tensor("wgc", [MP, P, 2, IO, P], fp8,
                         kind="ExternalInput").ap()
    wuc = nc.dram_tensor("wuc", [MP, P, 2, IO, P], fp8,
                         kind="ExternalInput").ap()
    wd4 = nc.dram_tensor("wd4", [4, P, 4, IO, P], fp8,
                         kind="ExternalInput").ap()
    y2 = nc.dram_tensor("y2", [HO // 2, P, 2, C], bf16,
                        kind="ExternalOutput").ap()

    with tile.TileContext(nc) as tc:
        with (
            tc.tile_pool(name="px", bufs=1) as px,
            tc.tile_pool(name="pwgc", bufs=MP) as pwgc,
            tc.tile_pool(name="pwuc", bufs=MP) as pwuc,
            tc.tile_pool(name="pwd", bufs=4) as pwd,
            tc.tile_pool(name="pmid", bufs=1) as pmid,
            tc.tile_pool(name="psg", bufs=IO) as psg,
            tc.tile_pool(name="py", bufs=4) as py,
            tc.tile_pool(name="pwrm", bufs=1) as pwrm,
            tc.tile_pool(name="pps", bufs=8, space="PSUM") as pps,
        ):
            # PE warmup: keep the tensor engine busy (and its clock ramping)
            # while the first input DMAs land. Short: the first gate chunk
            # plus x chunk arrive ~1.5us after the DMA kick.
            wtile = pwrm.tile([P, 512], bf16)
            nc.vector.memset(wtile[:], 0.0)
            pwm = pps.tile([P, 512], f32, name="acc")
            for w in range(5):
                nc.tensor.matmul(pwm[:], wtile[:, :P], wtile[:],
                                 start=(w == 0), stop=(w == 4))

            # All input DMAs ride the SP ring in strict consumption order:
            # gate chunks + x first (phase 1A is m-major and starts after
            # just wgc[0] + xT chunk 0), then up chunks, then wd.
            xt = px.tile([P, HO, C], fp8)
            wgc_t = [pwgc.tile([P, 2, IO, P], fp8, name=f"wgc{m}", tag="wgc")
                     for m in range(MP)]
            wuc_t = [pwuc.tile([P, 2, IO, P], fp8, name=f"wuc{m}", tag="wuc")
                     for m in range(MP)]
            wd_t = [pwd.tile([P, 4, IO, P], fp8, name=f"wd{h}", tag="wd")
                    for h in range(4)]

            XC = HO // 4  # xt in 4 chunks
            nc.sync.dma_start(wgc_t[0][:], wgc[0])
            nc.sync.dma_start(xt[:, 0:XC, :], xT[:, 0:XC, :])
            nc.sync.dma_start(wgc_t[1][:], wgc[1])
            nc.sync.dma_start(xt[:, XC:2 * XC, :], xT[:, XC:2 * XC, :])
            nc.sync.dma_start(wgc_t[2][:], wgc[2])
            nc.sync.dma_start(wgc_t[3][:], wgc[3])
            nc.sync.dma_start(xt[:, 2 * XC:3 * XC, :], xT[:, 2 * XC:3 * XC, :])
            nc.sync.dma_start(wgc_t[4][:], wgc[4])
            nc.sync.dma_start(wgc_t[5][:], wgc[5])
            nc.sync.dma_start(xt[:, 3 * XC:HO, :], xT[:, 3 * XC:HO, :])
            nc.sync.dma_start(wgc_t[6][:], wgc[6])
            nc.sync.dma_start(wgc_t[7][:], wgc[7])
            for m in range(MP):
                nc.sync.dma_start(wuc_t[m][:], wuc[m])
            for h in range(4):
                nc.sync.dma_start(wd_t[h][:], wd4[h])

            # Phase 1A, m-major over gate: 8 PSUM accumulators, one full
            # bank each, proper start/stop groups (one live group per bank).
            # The PE starts as soon as wgc[0] + xt chunk 0 land; the silus
            # pipeline behind the last h-pair sweep.
            mid = pmid.tile([P, IO, C], fp8)
            pg = [pps.tile([P, C], f32, name="acc") for _ in range(IO)]
            sgs = []
            for m in range(MP):
                for j in range(IO):
                    nc.tensor.matmul(
                        pg[j][:], wgc_t[m][:, :, j, :],
                        xt[:, 2 * m:2 * m + 2, :],
                        start=(m == 0), stop=(m == MP - 1),
                        perf_mode=DR,
                    )
                    if m == MP - 1:
                        sg = psg.tile([P, C], f32, name="sg")
                        nc.scalar.activation(sg[:], pg[j][:], SILU,
                                             scale=1.0 / SW)
                        sgs.append(sg)

            # Phase 1B, m-major over up: pu[j] reuses pg[j]'s bank after
            # its silu read. stt produces fp8 mid behind the last sweep.
            pu = [pps.tile([P, C], f32, name="acc") for _ in range(IO)]
            for m in range(MP):
                for j in range(IO):
                    nc.tensor.matmul(
                        pu[j][:], wuc_t[m][:, :, j, :],
                        xt[:, 2 * m:2 * m + 2, :],
                        start=(m == 0), stop=(m == MP - 1),
                        perf_mode=DR,
                    )
                    if m == MP - 1:
                        nc.vector.scalar_tensor_tensor(
                            out=mid[:, j, :], in0=pu[j][:], scalar=SM / SW,
                            in1=sgs[j][:], op0=MUL, op1=MUL,
                        )

            # Phase 2: y[k] = sum_j Wd[j, k].T @ mid[j] (i-tile pairs),
            # emitted in pairs of h-tiles per output DMA so the tail DMAs
            # stay small. pd banks rotate through the shared PSUM pool;
            # PSUM->SBUF casts alternate between the ACT and DVE engines and
            # the output DMA triggers alternate between the two idle rings.
            for q in range(HO // 2):
                ty = py.tile([P, 2, C], bf16)
                for kk in range(2):
                    k = 2 * q + kk
                    pd = pps.tile([P, C], f32, name="acc")
                    for j in range(IO // 2):
                        nc.tensor.matmul(
                            pd[:], wd_t[k // 4][:, k % 4, 2 * j:2 * j + 2, :],
                            mid[:, 2 * j:2 * j + 2, :],
                            start=(j == 0), stop=(j == IO // 2 - 1),
                            perf_mode=DR,
                        )
                    if kk == 0:
                        nc.scalar.activation(ty[:, kk, :], pd[:], COPY)
                    else:
                        nc.vector.tensor_copy(out=ty[:, kk, :], in_=pd[:])
                if q % 2 == 0:
                    nc.gpsimd.dma_start(y2[q], ty[:])
                else:
                    nc.sync.dma_start(y2[q], ty[:])

    nc.compile()
    return nc


def _route(x, router_w, corr_bias):
    """fp64 router: returns (topk_idx [T,K], topk_w [T,K])."""
    xl = x.astype(np.float64)
    logits = xl @ router_w.astype(np.float64).T
    logits -= logits.max(axis=1, keepdims=True)
    p = np.exp(logits)
    p /= p.sum(axis=1, keepdims=True)
    sel = p + corr_bias.astype(np.float64)
    topk_idx = np.argsort(-sel, axis=1, kind="stable")[:, :TOPK]
    topk_w = np.take_along_axis(p, topk_idx, axis=1) * ROUTED_SCALING
    return topk_idx, topk_w


def _pack_inputs(x8_toks, wg_e, wu_e, wd_e):
    """Device-layout packing for one expert: x8_toks [n<=C, H] fp8."""
    f8 = ml_dtypes.float8_e4m3

    def q8(a):
        return np.clip(a, -240.0, 240.0).astype(f8)

    n = len(x8_toks)
    xg = np.zeros((C, H), dtype=f8)
    xg[:n] = x8_toks
    xTd = np.ascontiguousarray(xg.T.reshape(HO, P, C).transpose(1, 0, 2))
    # [HO, P, IO, P] -> per-h-pair chunks [MP, P, 2(h), IO, P]
    wgcd = np.ascontiguousarray(
        q8(SW * wg_e).reshape(HO // 2, 2, P, IO, P).transpose(0, 2, 1, 3, 4))
    wucd = np.ascontiguousarray(
        q8(SW * wu_e).reshape(HO // 2, 2, P, IO, P).transpose(0, 2, 1, 3, 4))
    wdd = np.ascontiguousarray(
        q8(SW * wd_e).reshape(IO, P, HO, P)
        .transpose(2, 1, 0, 3)                 # [HO, P, IO, P]
        .reshape(4, 4, P, IO, P)
        .transpose(0, 2, 1, 3, 4))             # [4, P, 4, IO, P]
    return {"xT": xTd, "wgc": wgcd, "wuc": wucd, "wd4": wdd}


def kernel(hidden_states, router_w, corr_bias, w_gate, w_up, w_down):
    global _PROGRAM, LAST_RESULTS
    x = np.asarray(hidden_states, dtype=np.float32)
    router_w = np.asarray(router_w, dtype=np.float32)
    corr_bias = np.asarray(corr_bias, dtype=np.float32)
    w_gate = np.asarray(w_gate, dtype=np.float32)
    w_up = np.asarray(w_up, dtype=np.float32)
    w_down = np.asarray(w_down, dtype=np.float32)

    topk_idx, topk_w = _route(x, router_w, corr_bias)
    routed = topk_idx < E
    zero_w = (topk_w * (~routed)).sum(axis=1)  # [T] fp64

    f8 = ml_dtypes.float8_e4m3  # TRN fp8e4: e4m3 with max normal 240

    def q8(a):
        return np.clip(a, -240.0, 240.0).astype(f8)

    x8 = q8(x)

    # Dispatch: token list + gate weight per expert; overflow beyond C
    # falls back to an exact host computation.
    idx_list, w_list, overflow = [], [], []
    for e in range(E):
        toks, kpos = np.nonzero(topk_idx == e)
        we = topk_w[toks, kpos]
        if len(toks) > C:
            overflow.append((e, toks[C:], we[C:]))
            toks, we = toks[:C], we[:C]
        idx_list.append(toks)
        w_list.append(we)

    in_maps = [
        _pack_inputs(x8[idx_list[e]], w_gate[e], w_up[e], w_down[e])
        for e in range(E)
    ]

    if _PROGRAM is None:
        _PROGRAM = _build_program()

    from concourse.bass_utils import run_bass_kernel_spmd

    kw = {}
    if os.environ.get("MOE_KERNEL_TRACE", "") == "1":
        kw = dict(trace=True, trace_cores=list(range(N_CORES)))
    res = run_bass_kernel_spmd(
        _PROGRAM, in_maps, core_ids=list(range(N_CORES)), **kw)
    LAST_RESULTS = res

    out = np.zeros((T, H), dtype=np.float64)
    inv = 1.0 / (SW * SM)
    for e in range(E):
        n = len(idx_list[e])
        if n:
            y2e = res.results[e]["y2"]  # [HO/2, P, 2, C] bf16 = 256 * down
            ye = y2e.transpose(0, 2, 1, 3).reshape(H, C)
            out[idx_list[e]] += (w_list[e] * inv)[:, None] * \
                ye[:, :n].T.astype(np.float64)
    for e, toks, ws in overflow:
        xt = x[toks].astype(np.float64)
        g = xt @ w_gate[e].astype(np.float64)
        u = xt @ w_up[e].astype(np.float64)
        mid = (g / (1.0 + np.exp(-g))) * u
        out[toks] += ws[:, None] * (mid @ w_down[e].astype(np.float64))
    out += zero_w[:, None] * x.astype(np.float64)
    return out.astype(np.float32)
